# revision 1
# baseline (speedup 1.0000x reference)
"""Block attention (no softmax) Trainium2 Bass kernel.

Problem: x:[8,8192,128] -> q,k,v projections -> per-256-block attention with
a +/-255-row K/V window, NO softmax, -> out:[8,8192,128].

Key algebraic identity: with no softmax, (Q K^T * s) V == (Q * s) (K^T V).
Per window n, M_n = sum_{r in win(n)} k_r v_r^T is a [128,128] matrix; then
out_blk = (Q_blk * s) @ M_n.  This turns the [256x766] score matrices into
[128x128] K^T V accumulations, ~6x fewer FLOPs and no big score tensors.

Sharding: batch (8) across the 8 NeuronCores, data-parallel, no halo
exchange (windows never cross batch boundaries).

All matmul operands are fp16 (PSUM accumulates fp32; measured rel err
~4e-4 vs the fp32 reference).  x ships from the host already cast to fp16
(identical numerics to casting on device) so transposes run 1 cycle/row.

Engine layout per 512-row iteration, software-pipelined so the PE (warm
at 2.4 GHz once HAM engages) never starves:
  PE:   4 x-transposes, 12 window K^T V chunk matmuls (pair ci-2),
        4 out matmuls (pair ci-3), 1 qT (N=512), 4 k|v (N=256)
  DVE:  2 xT PSUM->SBUF copies, 2 k|v bias adds
  ACT:  qT bias+scale+cast, M cast, out-stage copy
  GPSIMD: zeroed-row0 k-chunk copies (PE base-partition workaround)
  Sync: x in-DMA, out DMA
"""

import sys
from contextlib import ExitStack

import numpy as np

for _p in ("/opt/trn_rl_repo", "/root/.axon_site/_ro/trn_rl_repo"):
    if _p not in sys.path:
        sys.path.append(_p)

import concourse.bass as bass
import concourse.tile as tile
from concourse import bacc, mybir
from concourse.bass_utils import run_bass_kernel_spmd

S = 8192          # sequence length per batch/core
D = 128           # input dim
H = 128           # hidden dim
BS = 256          # block size
HALO = 255        # window_size - 1
NB = S // BS      # 32 blocks
NCORES = 8
SCALE = float(1.0 / np.sqrt(np.float32(D)))

F32 = mybir.dt.float32
F16 = mybir.dt.float16
F32R = mybir.dt.float32r
CDT = F16  # matmul operand dtype (PSUM accumulation is always fp32)
AF = mybir.ActivationFunctionType


def _window_chunks(n):
    """128-aligned contraction chunks covering window n's valid rows.

    Window n covers rows [256n-255, 256n+511) clipped to [0, S).  All chunk
    starts are ==0 or ==1 (mod 128), so each chunk lives inside one
    128-partition group: returns (chunk_idx, p0, p1) triples.
    """
    lo = max(0, BS * n - HALO)
    hi = min(S, BS * n + BS + HALO)
    chunks = []
    a = lo
    while a < hi:
        b = min(hi, (a // 128 + 1) * 128)
        chunks.append((a // 128, a % 128, a % 128 + (b - a)))
        a = b
    return chunks


def build_nc():
    nc = bacc.Bacc(
        "TRN2",
        target_bir_lowering=False,
        debug=False,
        enable_asserts=False,
        num_devices=NCORES,
    )

    x = nc.dram_tensor("x", [S, D], CDT, kind="ExternalInput").ap()
    cf32 = nc.dram_tensor("cf32", [128, 513], F32, kind="ExternalInput").ap()
    cf16 = nc.dram_tensor("cf16", [128, 3 * H], CDT, kind="ExternalInput").ap()
    out = nc.dram_tensor("out", [S, H], F32, kind="ExternalOutput").ap()

    xv = x.rearrange("(c p) d -> p c d", p=128)       # [128, 64, 128]
    out_t = out.rearrange("(c p) h -> p c h", p=128)  # [128, 64, 128]

    with ExitStack() as ctx:
        tc = ctx.enter_context(tile.TileContext(nc))
        const = ctx.enter_context(tc.tile_pool(name="const", bufs=1))
        id_sb = const.tile([128, 128], CDT)
        nc.gpsimd.memset(id_sb, 1.0)
        nc.gpsimd.affine_select(
            id_sb, id_sb, [[1, 128]], mybir.AluOpType.is_equal, 0.0,
            base=0, channel_multiplier=-1,
        )
        cf32_sb = const.tile([128, 513], F32)
        cf16_sb = const.tile([128, 3 * H], CDT)
        bq_sb = cf32_sb[:, 0:1]
        bkv_sb = cf32_sb[:, 1:513].rearrange("p (a b) -> p a b", a=2)
        wq_sb = cf16_sb[:, 0:H]
        wkv_sb = cf16_sb[:, H : 3 * H]

        big = ctx.enter_context(tc.tile_pool(name="big", bufs=1))
        qT_all = big.tile([128, S], CDT)            # q^T, scaled, [h, s]
        kv_all = big.tile([128, S // 128, 2 * H], CDT)  # [p, chunk, k|v]
        # Copies of even k-chunks with row 0 zeroed: window head-chunks start
        # at partition 1, which the PE can't address (base partition must be
        # 0/32/64) — a zeroed row 0 contributes nothing to K^T V instead.
        kz_all = big.tile([128, 31, H], CDT)

        xn_pool = ctx.enter_context(tc.tile_pool(name="xn", bufs=6))
        xT_pool = ctx.enter_context(tc.tile_pool(name="xT", bufs=3))
        m_pool = ctx.enter_context(tc.tile_pool(name="m", bufs=4))
        o_pool = ctx.enter_context(tc.tile_pool(name="o", bufs=4))
        psum = ctx.enter_context(
            tc.tile_pool(name="ps", bufs=8, space=bass.MemorySpace.PSUM)
        )

        # ---- C+D emitter: window pair t covers out rows [512t, 512t+512).
        # Emitted inside the A+B loop (pair t needs kv chunks <= 4t+5 only),
        # so the PE always has independent window/out matmuls to hide the
        # transpose->cast->project dependency chain, and out DMAs spread
        # across the whole kernel instead of a tail.
        m2_tiles = {}

        def emit_c(t):
            psM = psum.tile([128, 2, 128], F32, tag="ps", name="psM")
            for w in range(2):
                n = 2 * t + w
                chunks = _window_chunks(n)
                for i, (c, p0, p1) in enumerate(chunks):
                    if p0 == 1:
                        # head chunk: zeroed-row0 copy, full 128 rows
                        lhs = kz_all[:, c // 2, :]
                        rhs = kv_all[:, c, H : 2 * H]
                    else:
                        lhs = kv_all[p0:p1, c, 0:H]
                        rhs = kv_all[p0:p1, c, H : 2 * H]
                    nc.tensor.matmul(
                        psM[:, w, :], lhs, rhs,
                        start=(i == 0),
                        stop=(i == len(chunks) - 1),
                    )
            m2 = m_pool.tile([128, 2, 128], CDT, tag="m")
            nc.scalar.copy(m2, psM)
            m2_tiles[t] = m2

        def emit_d(t):
            m2 = m2_tiles.pop(t)
            psO = psum.tile([128, 4, 128], F32, tag="ps", name="psO")
            for w in range(4):
                n, j = divmod(4 * t + w, 2)
                s0 = BS * n + 128 * j
                nc.tensor.matmul(
                    psO[:, w, :],
                    qT_all[:, s0 : s0 + 128],
                    m2[:, n - 2 * t, :],
                    start=True,
                    stop=True,
                )
            ostage = o_pool.tile([128, 4, 128], F32, tag="o")
            nc.scalar.copy(ostage, psO)
            nc.sync.dma_start(out_t[:, 4 * t : 4 * t + 4, :], ostage)

        # ---- PE warm-up: HAM needs ~3.4us of sustained PE activity to lift
        # the clock 1.2->2.4 GHz.  Burn dummy matmuls on scratch data during
        # the startup DMA window so the flip happens ~3us earlier.
        warm_sb = const.tile([128, 512], CDT)
        nc.gpsimd.memset(warm_sb, 0.0)
        psW = psum.tile([128, 512], F32, tag="ps", name="psW")
        for _ in range(5):
            nc.tensor.matmul(psW, id_sb, warm_sb, start=True, stop=True)

        # ---- Phase A+B: load x, cast, transpose, project q/k/v -------------
        for ci in range(S // 512):
            xn4 = xn_pool.tile([128, 4, 128], CDT, tag="xn")
            nc.sync.dma_start(xn4, xv[:, 4 * ci : 4 * ci + 4, :])
            xT = xT_pool.tile([128, 512], CDT, tag="xT")
            # two PSUM tiles so the first copy overlaps the later transposes
            # (same-bank PE-writes + DVE-reads would serialize)
            psA_a = psum.tile([128, 256], CDT, tag="ps", name="psA_a")
            psA_b = psum.tile([128, 256], CDT, tag="ps", name="psA_b")
            for j in range(4):
                pst = psA_a if j < 2 else psA_b
                nc.tensor.transpose(
                    pst[:, 128 * (j % 2) : 128 * (j % 2 + 1)],
                    xn4[:, j, :], id_sb,
                )
            nc.vector.tensor_copy(xT[:, 0:256], psA_a)
            nc.vector.tensor_copy(xT[:, 256:512], psA_b)

            if ci == 0:
                # defer non-identity consts until after the first transposes
                # so the first xn4 load isn't queued behind them
                nc.sync.dma_start(cf32_sb, cf32)
                nc.sync.dma_start(cf16_sb, cf16)

            # C/D matmuls of earlier window pairs fill the PE queue while
            # the xT copies (DVE) complete — PE is in-order per engine.
            if ci >= 2:
                emit_c(ci - 2)
            if ci >= 3:
                emit_d(ci - 3)

            def kv_pair(h):
                # k|v chunks: [s128, 256] = xT_j.T @ [wk_t | wv_t]; bias DVE
                psKV = psum.tile([128, 2, 2 * H], F32, tag="ps", name="psKV")
                for j2 in range(2):
                    j = 2 * h + j2
                    nc.tensor.matmul(
                        psKV[:, j2, :],
                        xT[:, 128 * j : 128 * (j + 1)],
                        wkv_sb,
                        start=True,
                        stop=True,
                    )
                cc = 4 * ci + 2 * h
                nc.vector.tensor_add(kv_all[:, cc : cc + 2, :], psKV, bkv_sb)
                if cc <= 60:
                    nc.gpsimd.tensor_copy(
                        kz_all[:, cc // 2, :], kv_all[:, cc, 0:H]
                    )
                    nc.gpsimd.memset(kz_all[0:1, cc // 2, :], 0.0)

            kv_pair(0)  # needs only xT cols 0:256 (first copy)

            # q^T chunk: [h, 512] = wq_t.T @ xT ; bias+scale fused on ACT copy
            psQ = psum.tile([128, 512], F32, tag="ps", name="psQ")
            nc.tensor.matmul(psQ, wq_sb, xT, start=True, stop=True)
            nc.scalar.activation(
                qT_all[:, 512 * ci : 512 * (ci + 1)],
                psQ,
                AF.Identity,
                bias=bq_sb,
                scale=SCALE,
            )

            kv_pair(1)

        emit_c(NB // 2 - 2)
        emit_d(NB // 2 - 3)
        emit_c(NB // 2 - 1)
        emit_d(NB // 2 - 2)
        emit_d(NB // 2 - 1)

    nc.compile()
    return nc


_NC_CACHE = None


def _get_nc():
    global _NC_CACHE
    if _NC_CACHE is None:
        _NC_CACHE = build_nc()
    return _NC_CACHE


def _make_in_maps(inputs):
    x = np.ascontiguousarray(np.asarray(inputs["x"], dtype=np.float32))
    Wq = np.asarray(inputs["Wq"], dtype=np.float32)
    Wk = np.asarray(inputs["Wk"], dtype=np.float32)
    Wv = np.asarray(inputs["Wv"], dtype=np.float32)
    bq = np.asarray(inputs["bq"], dtype=np.float32)
    bk = np.asarray(inputs["bk"], dtype=np.float32)
    bv = np.asarray(inputs["bv"], dtype=np.float32)

    wdt = np.float16 if CDT == F16 else np.float32
    cf16 = np.concatenate([Wq.T, Wk.T, Wv.T], axis=1).astype(wdt)
    # ACT computes func(in*scale + bias), so the q bias ships pre-scaled
    bq_col = (bq * SCALE).reshape(H, 1).astype(np.float32)
    bkv_row = np.concatenate([bk, bv])
    bkv_rep = np.broadcast_to(
        np.tile(bkv_row, 2)[None, :], (128, 4 * H)
    ).astype(np.float32)
    cf32 = np.concatenate([bq_col, bkv_rep], axis=1)

    shared = {
        "cf32": np.ascontiguousarray(cf32),
        "cf16": np.ascontiguousarray(cf16),
    }
    x16 = x.astype(np.float16) if CDT == F16 else x
    return [{"x": np.ascontiguousarray(x16[c]), **shared} for c in range(NCORES)]


def kernel(**inputs):
    nc = _get_nc()
    in_maps = _make_in_maps(inputs)
    res = run_bass_kernel_spmd(nc, in_maps, core_ids=list(range(NCORES)))
    return np.stack([res.results[c]["out"] for c in range(NCORES)], axis=0)


def run_traced(inputs):
    """Like kernel() but with NTFF tracing; returns (out, BassKernelResults)."""
    nc = _get_nc()
    in_maps = _make_in_maps(inputs)
    res = run_bass_kernel_spmd(
        nc, in_maps, core_ids=list(range(NCORES)), trace=True
    )
    out = np.stack([res.results[c]["out"] for c in range(NCORES)], axis=0)
    return out, res



# revision 6
# speedup vs baseline: 1.1011x; 1.1011x over previous
"""Block attention (no softmax) Trainium2 Bass kernel, v2.

Problem: x:[8,8192,128] -> q,k,v projections -> per-256-block attention with
a +/-255-row K/V window, NO softmax, -> out:[8,8192,128].

Key algebraic identity: with no softmax, (Q K^T * s) V == (Q * s) (K^T V).
Per window n, M_n = sum_{r in win(n)} k_r v_r^T is a [128,128] matrix; then
out_blk = (Q_blk * s) @ M_n.

Sharding: batch (8) across the 8 NeuronCores, data-parallel.

v2 structural changes over v1 (51.9us):
  * x is loaded TRANSPOSED via the DMA xbar (dma_start transpose=True) as
    one big xT_all[128, 8192] f16 tile -- eliminates 64 PE transposes and
    32 DVE PSUM->SBUF copies.  All 8 transpose loads are issued
    back-to-back (one xbar_mode run) right after the tiny const DMAs.
  * k/v chunks are sliced in PADDED window coordinates: chunk c holds orig
    rows [128c-255, 128c-127).  Window n = padded chunks 2n..2n+5, each a
    PREFIX of stored partitions -- every window matmul lhsT starts at
    partition 0, so the v1 zeroed-row-0 kz copies (22us of GpSimd) vanish.
    In-chunk placement doesn't matter: K^T V only needs k and v row-aligned.
  * f16 PSUM for all single-shot matmul outputs (kv, q, out) -- halves
    PSUM bank pressure; accumulated window PSUM (psM) stays f32.
  * Engine rebalance: DVE drains psKV->stage + m2 casts; GpSimd adds the
    k/v bias (SBUF-to-SBUF, was DVE-from-PSUM); ACT drains qT (+bias+scale
    fused) and out-stage; drains merged to 2-ci/psum-bank granularity.
  * HAM warm-up: 9 dummy N=512 matmuls cover the full free-running 3.4us
    activity window so the 1.2->2.4 GHz flip happens at ~3.4us, not 13.6us.
"""

import sys
from contextlib import ExitStack

import numpy as np

for _p in ("/opt/trn_rl_repo", "/root/.axon_site/_ro/trn_rl_repo"):
    if _p not in sys.path:
        sys.path.append(_p)

import concourse.bass as bass
import concourse.tile as tile
from concourse import bacc, mybir
from concourse.bass_utils import run_bass_kernel_spmd

S = 8192          # sequence length per batch/core
D = 128           # input dim
H = 128           # hidden dim
BS = 256          # block size
HALO = 255        # window_size - 1
NB = S // BS      # 32 blocks
NCORES = 8
SCALE = float(1.0 / np.sqrt(np.float32(D)))

F32 = mybir.dt.float32
F16 = mybir.dt.float16
CDT = F16
AF = mybir.ActivationFunctionType

WARMUP_MMS = 9    # dummy N=512 matmuls ~= 3.8us at cold 1.2 GHz


def _chunk_stored(c):
    """Stored row count of padded chunk c (orig rows [128c-255, 128c-127))."""
    return min(S, 128 * c - 127) - max(0, 128 * c - 255)


def _win_chunks(n):
    """(chunk, prefix_rows) pairs for window n = padded chunks 2n..2n+5.

    Window n covers padded rows [256n, 256n+766); chunk 2n+5 is clipped to
    its first 126 rows.  Stored rows are always a prefix from partition 0.
    """
    out = []
    for c in range(2 * n, 2 * n + 6):
        p = _chunk_stored(c)
        if c == 2 * n + 5:
            p = min(126, p)
        if p > 0:
            out.append((c, p))
    return out


def build_nc():
    nc = bacc.Bacc(
        "TRN2",
        target_bir_lowering=False,
        debug=False,
        enable_asserts=False,
        num_devices=NCORES,
    )

    x = nc.dram_tensor("x", [S, D], CDT, kind="ExternalInput").ap()
    cw = nc.dram_tensor("cw", [128, 3 * H], CDT, kind="ExternalInput").ap()
    cb = nc.dram_tensor("cb", [1, 2 * 2 * H], CDT, kind="ExternalInput").ap()
    cq = nc.dram_tensor("cq", [128, 1], F32, kind="ExternalInput").ap()
    out = nc.dram_tensor("out", [S, H], CDT, kind="ExternalOutput").ap()

    out_t = out.rearrange("(c p) h -> p c h", p=128)  # [128, 64, 128]

    with ExitStack() as ctx:
        tc = ctx.enter_context(tile.TileContext(nc))
        const = ctx.enter_context(tc.tile_pool(name="const", bufs=1))
        cw_sb = const.tile([128, 3 * H], CDT)
        wq_sb = cw_sb[:, 0:H]
        wkv_sb = cw_sb[:, H : 3 * H]
        bq_sb = const.tile([128, 1], F32)
        cbrow = const.tile([1, 4 * H], CDT)
        bkv2_sb = const.tile([128, 2, 2 * H], CDT)
        warm_sb = const.tile([128, 512], CDT)

        big = ctx.enter_context(tc.tile_pool(name="big", bufs=1))
        xT_all = big.tile([128, S], CDT)                 # x^T  [d, s]
        qT_all = big.tile([128, S], CDT)                 # q^T, scaled [h, s]
        kv_all = big.tile([128, 66, 2 * H], CDT)         # padded chunks [p,c,k|v]

        m_pool = ctx.enter_context(tc.tile_pool(name="m", bufs=2))
        o_pool = ctx.enter_context(tc.tile_pool(name="o", bufs=2))
        psum = ctx.enter_context(
            tc.tile_pool(name="ps", bufs=8, space=bass.MemorySpace.PSUM)
        )

        # ---- consts + PE warm-up (HAM needs ~3.4us of sustained PE
        # activity from t=0 to lift the clock 1.2->2.4 GHz at the earliest
        # possible ~3.4us mark).
        nc.gpsimd.memset(warm_sb, 0.0)
        nc.sync.dma_start(cw_sb, cw)
        nc.sync.dma_start(bq_sb, cq)
        nc.sync.dma_start(cbrow, cb)
        nc.gpsimd.partition_broadcast(bkv2_sb, cbrow)

        psW = psum.tile([128, 512], F32, tag="ps", name="psW")
        for _ in range(WARMUP_MMS):
            nc.tensor.matmul(psW, warm_sb[:, 0:128], warm_sb, start=True, stop=True)

        # ---- x transposed loads: 8 x [1024,128]->[128,1024] via DMA xbar,
        # all consecutive (one xbar_mode run; Tile serializes transitions).
        for L in range(8):
            nc.sync.dma_start(
                xT_all[:, 1024 * L : 1024 * (L + 1)],
                x[1024 * L : 1024 * (L + 1), :],
                transpose=True,
            )

        # ---- emitters ------------------------------------------------------
        # kv projection: chunk c stores orig rows [128c-255, 128c-127) at
        # partitions [0:stored).  lhsT = xT columns of those rows; chunk 1
        # is computed full-width (xT[:,0:128] -> row r at partition r) so
        # partition 0 = row 0, the only row windows read from it.
        def kv_chunk_mm(psKV, slot, c):
            if c == 1:
                lhsT = xT_all[:, 0:128]
                dst = psKV[:, slot, :]
            else:
                a = 128 * c - 255
                p = _chunk_stored(c)
                lhsT = xT_all[:, a : a + p]
                dst = psKV[0:p, slot, :]
            nc.tensor.matmul(dst, lhsT, wkv_sb, start=True, stop=True)

        m2_tiles = {}

        def emit_c2(u):
            """K^T V for windows 4u..4u+3 -> psM4 (f32) -> m2 (f16)."""
            psM4 = psum.tile([128, 4, 128], F32, tag="ps", name="psM4")
            for w in range(4):
                n = 4 * u + w
                chunks = _win_chunks(n)
                for i, (c, p) in enumerate(chunks):
                    nc.tensor.matmul(
                        psM4[:, w, :],
                        kv_all[0:p, c, 0:H],
                        kv_all[0:p, c, H : 2 * H],
                        start=(i == 0),
                        stop=(i == len(chunks) - 1),
                    )
            m2 = m_pool.tile([128, 4, 128], CDT, tag="m")
            nc.scalar.copy(m2, psM4)
            m2_tiles[u] = m2

        def emit_d2(u, split_dma=False):
            """out rows [1024u, 1024u+1024) = (Q*s) @ M -> DRAM."""
            m2 = m2_tiles.pop(u)
            ostage = o_pool.tile([128, 8, 128], CDT, tag="o")
            for half in range(2):
                psO = psum.tile([128, 4, 128], F32, tag="ps", name="psO")
                for w4 in range(4):
                    w = 4 * half + w4
                    n = 4 * u + w // 2
                    s0 = BS * n + 128 * (w % 2)
                    nc.tensor.matmul(
                        psO[:, w4, :],
                        qT_all[:, s0 : s0 + 128],
                        m2[:, w // 2, :],
                        start=True,
                        stop=True,
                    )
                dst = ostage[:, 4 * half : 4 * half + 4, :]
                if half == 0:
                    nc.scalar.copy(dst, psO)
                else:
                    nc.vector.tensor_copy(dst, psO)
            if split_dma:
                nc.sync.dma_start(out_t[:, 8 * u : 8 * u + 4, :], ostage[:, 0:4, :])
                nc.sync.dma_start(out_t[:, 8 * u + 4 : 8 * u + 8, :], ostage[:, 4:8, :])
            else:
                nc.sync.dma_start(out_t[:, 8 * u : 8 * u + 8, :], ostage)

        # ---- main loop: ci = 512-col stripe of xT --------------------------
        # PE order per ci (steady state):
        #   odd ci:  kv pair0 | emit_c2((ci-3)//2) | kv pair1 | q
        #   even ci: kv pair0 | emit_d2((ci-4)//2) | kv pair1 | q
        # c2 uses only chunks <= 4(ci-1)+4 (previous ci's bias-add had a full
        # ci to drain); d2 uses m2 from the previous ci and qT group u<=
        # (ci-4)//2 (its ACT ran at ci-1 or earlier).
        for ci in range(16):
            # kv chunks drain straight from PSUM with the bias TT (DVE;
            # GPSIMD has no PSUM access on TRN2).
            tt_eng = nc.vector

            psKVa = psum.tile([128, 2, 2 * H], F32, tag="ps", name="psKVa")
            kv_chunk_mm(psKVa, 0, 4 * ci + 1)
            kv_chunk_mm(psKVa, 1, 4 * ci + 2)
            tt_eng.tensor_add(
                kv_all[:, 4 * ci + 1 : 4 * ci + 3, :], psKVa, bkv2_sb
            )

            if ci >= 3 and ci % 2 == 1:
                emit_c2((ci - 3) // 2)
            if ci >= 4 and ci % 2 == 0:
                emit_d2((ci - 4) // 2)

            psKVb = psum.tile([128, 2, 2 * H], F32, tag="ps", name="psKVb")
            kv_chunk_mm(psKVb, 0, 4 * ci + 3)
            kv_chunk_mm(psKVb, 1, 4 * ci + 4)
            tt_eng.tensor_add(
                kv_all[:, 4 * ci + 3 : 4 * ci + 5, :], psKVb, bkv2_sb
            )

            # q^T stripe: [h, 512] = wq^T.T @ xT ; bias+scale fused on ACT
            psQ = psum.tile([128, 512], F32, tag="ps", name="psQ")
            nc.tensor.matmul(
                psQ,
                wq_sb,
                xT_all[:, 512 * ci : 512 * (ci + 1)],
                start=True,
                stop=True,
            )
            nc.scalar.activation(
                qT_all[:, 512 * ci : 512 * (ci + 1)],
                psQ,
                AF.Identity,
                bias=bq_sb,
                scale=SCALE,
            )

        # ---- epilogue: chunk 65, then the last window/out groups ----------
        psKV65 = psum.tile([128, 2 * H], F32, tag="ps", name="psKV65")
        p65 = _chunk_stored(65)  # 127
        nc.tensor.matmul(
            psKV65[0:p65, :], xT_all[:, 8065 : 8065 + p65], wkv_sb,
            start=True, stop=True,
        )
        nc.vector.tensor_add(
            kv_all[0:p65, 65, :], psKV65[0:p65, :], bkv2_sb[0:p65, 0, :]
        )

        emit_c2(6)
        emit_d2(6)
        emit_c2(7)
        emit_d2(7, split_dma=True)

    nc.compile()
    return nc


_NC_CACHE = None


def _get_nc():
    global _NC_CACHE
    if _NC_CACHE is None:
        _NC_CACHE = build_nc()
    return _NC_CACHE


def _make_in_maps(inputs):
    x = np.ascontiguousarray(np.asarray(inputs["x"], dtype=np.float32))
    Wq = np.asarray(inputs["Wq"], dtype=np.float32)
    Wk = np.asarray(inputs["Wk"], dtype=np.float32)
    Wv = np.asarray(inputs["Wv"], dtype=np.float32)
    bq = np.asarray(inputs["bq"], dtype=np.float32)
    bk = np.asarray(inputs["bk"], dtype=np.float32)
    bv = np.asarray(inputs["bv"], dtype=np.float32)

    cw = np.concatenate([Wq.T, Wk.T, Wv.T], axis=1).astype(np.float16)
    # ACT computes func(in*scale + bias), so the q bias ships pre-scaled
    cq = (bq * SCALE).reshape(H, 1).astype(np.float32)
    cb = np.tile(np.concatenate([bk, bv]), 2)[None, :].astype(np.float16)

    shared = {
        "cw": np.ascontiguousarray(cw),
        "cb": np.ascontiguousarray(cb),
        "cq": np.ascontiguousarray(cq),
    }
    x16 = x.astype(np.float16)
    return [{"x": np.ascontiguousarray(x16[c]), **shared} for c in range(NCORES)]


def kernel(**inputs):
    nc = _get_nc()
    in_maps = _make_in_maps(inputs)
    res = run_bass_kernel_spmd(nc, in_maps, core_ids=list(range(NCORES)))
    return np.stack(
        [res.results[c]["out"] for c in range(NCORES)], axis=0
    ).astype(np.float32)


def run_traced(inputs):
    """Like kernel() but with NTFF tracing; returns (out, BassKernelResults)."""
    nc = _get_nc()
    in_maps = _make_in_maps(inputs)
    res = run_bass_kernel_spmd(
        nc, in_maps, core_ids=list(range(NCORES)), trace=True
    )
    out = np.stack(
        [res.results[c]["out"] for c in range(NCORES)], axis=0
    ).astype(np.float32)
    return out, res
